# revision 1
# baseline (speedup 1.0000x reference)
"""Trainium2 Bass kernel for the layered-circuit WMC problem.

Computation (see reference): vals = [weights | neg_weights]  # [B, 8192]
12 alternating AND(prod)/OR(sum) layers, each gathering 2 children per node
from the previous layer's 8192 node values, then a final sum over nodes.

Sharding: data-parallel over batch; 8 cores x 128 batch rows each.

Device design (per core):
- Node values live in HBM node-major: V[node, 128 batch] fp32 (512B rows).
- Per layer: two `dma_gather`s (SWDGE descriptor gather, 8192 rows each)
  pull child0/child1 rows into SBUF as [128, 64, 128]; the Vector engine
  combines (mult for AND layers / add for OR layers); an HWDGE dma_start
  writes the result back to HBM for the next layer's gather.
- The write-back stores node j at row (j%128)*64 + j//128, which makes each
  SBUF partition's 32KB contiguous (128 big descriptors instead of 8192
  small ones). The next layer's index lists are premapped through that
  permutation on the host.
- The last OR layer + root sum collapse into a count-weighted reduction
  (host precomputes how often each layer-10 node appears in layer 11):
  64 accumulated PE matmuls against the count vector -> [1, 128] per core.

The compiled NEFF is input-independent (indices are runtime data) and is
cached across calls.
"""

import numpy as np

N_LAYERS = 12
DEV_LAYERS = 11  # layers 0..10 on device; layer 11 + root folded into counts
WIDTH = 8192
N_VARS = 4096
BATCH = 1024
N_CORES = 8
PB = BATCH // N_CORES  # 128 batch rows per core
CH = WIDTH // 128  # 64 chunks
IDXF = WIDTH // 16  # 512 int16 per partition per gather list

_CACHE = {}


def _build_nc():
    import concourse.bacc as bacc
    import concourse.mybir as mybir

    f32 = mybir.dt.float32
    i16 = mybir.dt.int16

    nc = bacc.Bacc("TRN2", target_bir_lowering=False, debug=False)

    v0 = nc.dram_tensor("v0", [WIDTH, PB], f32, kind="ExternalInput")
    idxs = nc.dram_tensor(
        "idxs", [PB, 2 * DEV_LAYERS * IDXF], i16, kind="ExternalInput"
    )
    cnt = nc.dram_tensor("cnt", [PB, CH], f32, kind="ExternalInput")
    out = nc.dram_tensor("out", [1, PB], f32, kind="ExternalOutput")

    va = nc.dram_tensor("va", [WIDTH, PB], f32)
    vb = nc.dram_tensor("vb", [WIDTH, PB], f32)
    vp = [va, vb]

    def src_ap(l):
        return v0[:] if l == 0 else vp[(l + 1) % 2][:]

    def dst_ap(l):  # write-back target of layer l
        return vp[l % 2][:].rearrange("(p c) e -> p c e", p=PB, c=CH)

    with (
        nc.sbuf_tensor("g0", [PB, CH, 128], f32) as g0,
        nc.sbuf_tensor("g1", [PB, CH, 128], f32) as g1,
        nc.sbuf_tensor("comb", [PB, CH, 128], f32) as comb,
        nc.sbuf_tensor("idx_sb", [PB, 2 * DEV_LAYERS * IDXF], i16) as idx_sb,
        nc.sbuf_tensor("cnt_sb", [PB, CH], f32) as cnt_sb,
        nc.sbuf_tensor("res", [1, PB], f32) as res,
        nc.psum_tensor("ps", [1, PB], f32) as ps,
        nc.semaphore("io") as io,
        nc.semaphore("gsem") as gsem,
        nc.semaphore("csem") as csem,
        nc.semaphore("wsem") as wsem,
        nc.semaphore("psem") as psem,
        nc.Block() as block,
    ):

        @block.gpsimd
        def _(g):
            from concourse import library_config

            g.load_library(library_config.mlp)
            g.wait_ge(io, 32)  # idx list + cnt loaded
            for l in range(DEV_LAYERS):
                if l > 0:
                    g.wait_ge(wsem, 16 * l)  # V_l written back
                    g.wait_ge(csem, l)  # g0/g1 free (combine l-1 done)
                g.dma_gather(
                    g0[:], src_ap(l), idx_sb[:, (2 * l) * IDXF : (2 * l + 1) * IDXF],
                    WIDTH, WIDTH, 128, single_packet=False,
                ).then_inc(gsem, 16)
                g.dma_gather(
                    g1[:], src_ap(l), idx_sb[:, (2 * l + 1) * IDXF : (2 * l + 2) * IDXF],
                    WIDTH, WIDTH, 128, single_packet=False,
                ).then_inc(gsem, 16)

        @block.vector
        def _(v):
            for l in range(DEV_LAYERS):
                v.wait_ge(gsem, 32 * (l + 1))
                if l > 0:
                    v.wait_ge(wsem, 16 * l)  # comb free (write-back l-1 done)
                op = mybir.AluOpType.mult if l % 2 == 0 else mybir.AluOpType.add
                v.tensor_tensor(out=comb[:], in0=g0[:], in1=g1[:], op=op).then_inc(
                    csem, 1
                )
            v.wait_ge(psem, 1)
            v.tensor_copy(out=res[:], in_=ps[:]).then_inc(csem, 1)

        @block.sync
        def _(s):
            s.dma_start(idx_sb[:], idxs[:]).then_inc(io, 16)
            s.dma_start(cnt_sb[:], cnt[:]).then_inc(io, 16)
            for l in range(DEV_LAYERS - 1):
                s.wait_ge(csem, l + 1)
                s.dma_start(dst_ap(l), comb[:]).then_inc(wsem, 16)
            s.wait_ge(csem, DEV_LAYERS + 1)
            s.dma_start(out[:], res[:]).then_inc(io, 16)
            s.wait_ge(io, 48)

        @block.tensor
        def _(t):
            t.wait_ge(io, 32)  # cnt loaded
            t.wait_ge(csem, DEV_LAYERS)  # comb = layer-10 values
            for c in range(CH):
                mm = t.matmul(
                    ps[:],
                    cnt_sb[:, c : c + 1],
                    comb[:, c, :],
                    start=(c == 0),
                    stop=(c == CH - 1),
                )
            mm.then_inc(psem, 1)

    nc.compile()
    return nc


def _get_nc():
    if "nc" not in _CACHE:
        _CACHE["nc"] = _build_nc()
    return _CACHE["nc"]


def _wrap_idx(idx_list):
    """int16 wrapped layout: index j -> partition j%16 (replicated across the
    8 Q7 cores), int16 free position j//16."""
    return np.tile(idx_list.reshape(-1, 16).T, (8, 1)).astype(np.int16)


def _prep_inputs(weights, neg_weights, children):
    w = np.asarray(weights, np.float32)
    nw = np.asarray(neg_weights, np.float32)
    ch = np.asarray(children, np.int64)

    leaves = np.concatenate([w, nw], axis=1)  # [1024, 8192]

    # write-back permutation: original node j -> row (j%128)*64 + j//128
    def perm(j):
        return (j % 128) * CH + j // 128

    idx_blocks = []
    for l in range(DEV_LAYERS):
        c0, c1 = ch[l, :, 0], ch[l, :, 1]
        if l > 0:
            c0, c1 = perm(c0), perm(c1)
        idx_blocks.append(_wrap_idx(c0.astype(np.int16)))
        idx_blocks.append(_wrap_idx(c1.astype(np.int16)))
    idx_arr = np.ascontiguousarray(np.concatenate(idx_blocks, axis=1))

    # layer-11 counts over layer-10 outputs, in comb[p, c] layout (j = c*128+p)
    count11 = np.bincount(ch[11].ravel(), minlength=WIDTH).astype(np.float32)
    cnt_pc = np.ascontiguousarray(count11.reshape(CH, 128).T)  # [128, 64]

    in_maps = []
    for c in range(N_CORES):
        v0 = np.ascontiguousarray(leaves[c * PB : (c + 1) * PB].T)  # [8192, 128]
        in_maps.append({"v0": v0, "idxs": idx_arr, "cnt": cnt_pc})
    return in_maps


def run(weights, neg_weights, children, trace=False):
    from concourse.bass_utils import run_bass_kernel_spmd

    nc = _get_nc()
    in_maps = _prep_inputs(weights, neg_weights, children)
    br = run_bass_kernel_spmd(nc, in_maps, list(range(N_CORES)), trace=trace)
    out = np.concatenate([r["out"][0] for r in br.results]).astype(np.float32)
    return out, br


def kernel(weights, neg_weights, children):
    out, _ = run(weights, neg_weights, children)
    return out



# revision 8
# speedup vs baseline: 3.8391x; 3.8391x over previous
"""v2-DRAM: baseline design (fp32, DRAM node-major values, non-transpose
SWDGE gathers, HWDGE write-back, count-folded final layer) with descriptor
generation split across 4 SWDGE queues (4 gathers of 4096 idx per layer,
one per Q7 core pair) instead of 2 gathers of 8192 on queue 0.
"""

import numpy as np

N_LAYERS = 12
DEV_LAYERS = 11
WIDTH = 8192
N_VARS = 4096
BATCH = 1024
N_CORES = 8
PB = BATCH // N_CORES
CH = WIDTH // 128
HALF = WIDTH // 2
IDXC = HALF // 16  # 256 int16 per partition per half-list

_CACHE = {}


def _build_nc():
    import concourse.bacc as bacc
    import concourse.mybir as mybir

    f32 = mybir.dt.float32
    i16 = mybir.dt.int16

    nc = bacc.Bacc(
        "TRN2",
        target_bir_lowering=False,
        debug=False,
        num_swdge_queues=4,
        dynamic_dma_scratch_size=32768,
    )

    v0 = nc.dram_tensor("v0", [WIDTH, PB], f32, kind="ExternalInput")
    idxs = nc.dram_tensor("idxs", [128, DEV_LAYERS * 4 * IDXC], i16, kind="ExternalInput")
    cnt = nc.dram_tensor("cnt", [PB, CH], f32, kind="ExternalInput")
    out = nc.dram_tensor("out", [1, PB], f32, kind="ExternalOutput")

    va = nc.dram_tensor("va", [WIDTH, PB], f32)
    vb = nc.dram_tensor("vb", [WIDTH, PB], f32)
    vp = [va, vb]

    def src_ap(l):
        return v0[:] if l == 0 else vp[(l + 1) % 2][:]

    def dst_ap(l):  # write-back target of layer l
        return vp[l % 2][:].rearrange("(p c) e -> p c e", p=PB, c=CH)

    with (
        nc.sbuf_tensor("g0", [PB, CH, 128], f32) as g0,
        nc.sbuf_tensor("g1", [PB, CH, 128], f32) as g1,
        nc.sbuf_tensor("comb", [PB, CH, 128], f32) as comb,
        nc.sbuf_tensor("idx_sb", [128, DEV_LAYERS * 4 * IDXC], i16) as idx_sb,
        nc.sbuf_tensor("cnt_sb", [PB, CH], f32) as cnt_sb,
        nc.sbuf_tensor("res", [1, PB], f32) as res,
        nc.psum_tensor("ps", [1, PB], f32) as ps,
        nc.semaphore("io") as io,
        nc.semaphore("dsem0") as dsem0,
        nc.semaphore("dsem1") as dsem1,
        nc.semaphore("dsem2") as dsem2,
        nc.semaphore("dsem3") as dsem3,
        nc.semaphore("csem") as csem,
        nc.semaphore("wsem") as wsem,
        nc.semaphore("psem") as psem,
        nc.Block() as block,
    ):
        dsems = [dsem0, dsem1, dsem2, dsem3]

        def gather_args(l):
            """(dst_ap, idx_ap) x4: child0/child1 halves of layer l.

            Non-transpose gather with num_idxs=4096 writes token t of its
            list at [t%128, t//128, :], so half h of list f lands in
            g{f}[:, h*32:(h+1)*32, :]."""
            base = l * 4 * IDXC
            outs = []
            for f, dst in enumerate((g0, g1)):
                for h in range(2):
                    d = dst[:, h * 32 : (h + 1) * 32, :]
                    i = idx_sb[:, base + (2 * f + h) * IDXC : base + (2 * f + h + 1) * IDXC]
                    outs.append((d, i))
            return outs

        @block.gpsimd
        def _(g):
            from concourse import library_config

            g.load_library(library_config.mlp)
            g.wait_ge(io, 32)  # idx + cnt loaded (v0 is ExternalInput)
            for l in range(DEV_LAYERS):
                if l > 0:
                    g.wait_ge(wsem, 16 * l)  # V_l written back
                    g.wait_ge(csem, l)  # g0/g1 free (combine l-1 done)
                for q, (d, i) in enumerate(gather_args(l)):
                    g.dma_gather(
                        d, src_ap(l), i, HALF, HALF, 128,
                        single_packet=False,
                        queue_num=q,
                    ).then_inc(dsems[q], 16)

        mult = mybir.AluOpType.mult
        add = mybir.AluOpType.add

        @block.vector
        def _(v):
            for l in range(DEV_LAYERS):
                for q in range(4):
                    v.wait_ge(dsems[q], 16 * (l + 1))
                if l > 0:
                    v.wait_ge(wsem, 16 * l)  # comb free (write-back l-1 done)
                op = mult if l % 2 == 0 else add
                v.tensor_tensor(out=comb[:], in0=g0[:], in1=g1[:], op=op).then_inc(
                    csem, 1
                )
            v.wait_ge(psem, 1)
            v.tensor_copy(out=res[:], in_=ps[:]).then_inc(csem, 1)

        @block.sync
        def _(s):
            s.dma_start(idx_sb[:], idxs[:]).then_inc(io, 16)
            s.dma_start(cnt_sb[:], cnt[:]).then_inc(io, 16)
            s.wait_ge(io, 32)
            for l in range(DEV_LAYERS - 1):
                s.wait_ge(csem, l + 1)
                s.dma_start(dst_ap(l), comb[:]).then_inc(wsem, 16)
            s.wait_ge(csem, DEV_LAYERS + 1)
            s.dma_start(out[:], res[:]).then_inc(io, 16)
            s.wait_ge(io, 48)

        @block.tensor
        def _(t):
            t.wait_ge(io, 32)  # cnt loaded
            t.wait_ge(csem, DEV_LAYERS)  # comb = layer-10 values
            for c in range(CH):
                mm = t.matmul(
                    ps[:],
                    cnt_sb[:, c : c + 1],
                    comb[:, c, :],
                    start=(c == 0),
                    stop=(c == CH - 1),
                )
            mm.then_inc(psem, 1)

    nc.compile()
    return nc


def _get_nc():
    if "nc" not in _CACHE:
        _CACHE["nc"] = _build_nc()
    return _CACHE["nc"]


def _wrap_idx(idx_list):
    return np.tile(idx_list.reshape(-1, 16).T, (8, 1)).astype(np.int16)


def _prep_inputs(weights, neg_weights, children):
    w = np.asarray(weights, np.float32)
    nw = np.asarray(neg_weights, np.float32)
    ch = np.asarray(children, np.int64)

    leaves = np.concatenate([w, nw], axis=1)  # [1024, 8192]

    # write-back permutation: original node j -> row (j%128)*64 + j//128
    def perm(j):
        return (j % 128) * CH + j // 128

    idx_blocks = []
    for l in range(DEV_LAYERS):
        for f in range(2):
            cl = ch[l, :, f]
            if l > 0:
                cl = perm(cl)
            cl = cl.astype(np.int16)
            idx_blocks.append(_wrap_idx(cl[:HALF]))
            idx_blocks.append(_wrap_idx(cl[HALF:]))
    idx_arr = np.ascontiguousarray(np.concatenate(idx_blocks, axis=1))

    count11 = np.bincount(ch[11].ravel(), minlength=WIDTH).astype(np.float32)
    cnt_pc = np.ascontiguousarray(count11.reshape(CH, 128).T)  # [128, 64]

    in_maps = []
    for c in range(N_CORES):
        v0c = np.ascontiguousarray(leaves[c * PB : (c + 1) * PB].T)  # [8192, 128]
        in_maps.append({"v0": v0c, "idxs": idx_arr, "cnt": cnt_pc})
    return in_maps


def run(weights, neg_weights, children, trace=False):
    from concourse.bass_utils import run_bass_kernel_spmd

    nc = _get_nc()
    in_maps = _prep_inputs(weights, neg_weights, children)
    br = run_bass_kernel_spmd(nc, in_maps, list(range(N_CORES)), trace=trace)
    out = np.concatenate([r["out"][0] for r in br.results]).astype(np.float32)
    return out, br


def kernel(weights, neg_weights, children):
    out, _ = run(weights, neg_weights, children)
    return out


# revision 9
# speedup vs baseline: 5.3093x; 1.3829x over previous
"""v2.5: v2dram (fp32 DRAM values, 4-queue non-transpose gathers) plus
prepare_only/trigger pipelining — descriptors for layer l+1 are generated
on the Q7 cores while layer l's DMA drain / combine / write-back run — and
half-split combines and write-backs so the serial tail between descgen
phases shrinks.
"""

import numpy as np

N_LAYERS = 12
DEV_LAYERS = 11
WIDTH = 8192
N_VARS = 4096
BATCH = 1024
N_CORES = 8
PB = BATCH // N_CORES
CH = WIDTH // 128
HALF = WIDTH // 2
IDXC = HALF // 16

_CACHE = {}


def _build_nc():
    import concourse.bacc as bacc
    import concourse.mybir as mybir

    f32 = mybir.dt.float32
    i16 = mybir.dt.int16

    nc = bacc.Bacc(
        "TRN2",
        target_bir_lowering=False,
        debug=False,
        num_swdge_queues=4,
        dynamic_dma_scratch_size=49152,
    )

    v0 = nc.dram_tensor("v0", [WIDTH, PB], f32, kind="ExternalInput")
    idxs = nc.dram_tensor("idxs", [128, DEV_LAYERS * 4 * IDXC], i16, kind="ExternalInput")
    cnt = nc.dram_tensor("cnt", [PB, CH], f32, kind="ExternalInput")
    out = nc.dram_tensor("out", [1, PB], f32, kind="ExternalOutput")

    va = nc.dram_tensor("va", [WIDTH, PB], f32)
    vb = nc.dram_tensor("vb", [WIDTH, PB], f32)
    vp = [va, vb]

    def src_ap(l):
        return v0[:] if l == 0 else vp[(l + 1) % 2][:]

    def dst_ap(l, s):  # write-back target of layer l, half s (chunks 32s..32s+31)
        full = vp[l % 2][:].rearrange("(p c) e -> p c e", p=PB, c=CH)
        return full[:, 32 * s : 32 * (s + 1), :]

    with (
        nc.sbuf_tensor("g0", [PB, CH, 128], f32) as g0,
        nc.sbuf_tensor("g1", [PB, CH, 128], f32) as g1,
        nc.sbuf_tensor("comb", [PB, CH, 128], f32) as comb,
        nc.sbuf_tensor("idx_sb", [128, DEV_LAYERS * 4 * IDXC], i16) as idx_sb,
        nc.sbuf_tensor("cnt_sb", [PB, CH], f32) as cnt_sb,
        nc.sbuf_tensor("res", [1, PB], f32) as res,
        nc.psum_tensor("ps", [1, PB], f32) as ps,
        nc.semaphore("io") as io,
        nc.semaphore("dsem0") as dsem0,
        nc.semaphore("dsem1") as dsem1,
        nc.semaphore("dsem2") as dsem2,
        nc.semaphore("dsem3") as dsem3,
        nc.semaphore("prepsem") as prepsem,
        nc.semaphore("csem") as csem,  # 2 per layer (half combines) + 1 final
        nc.semaphore("wsemA") as wsemA,  # 16 per layer (half-A write-back)
        nc.semaphore("wsemB") as wsemB,  # 16 per layer (half-B write-back)
        nc.semaphore("psem") as psem,
        nc.Block() as block,
    ):
        dsems = [dsem0, dsem1, dsem2, dsem3]

        def gather_args(l):
            """(queue, dst_ap, idx_ap) x4 for layer l.

            q0: g0 slots 0..4095 (chunks 0..31), q1: g0 slots 4096.. (32..63),
            q2: g1 lo, q3: g1 hi. Half-set A = {q0, q2} covers comb chunks
            0..31; set B = {q1, q3} covers 32..63."""
            base = l * 4 * IDXC
            outs = []
            for f, dst in enumerate((g0, g1)):
                for h in range(2):
                    d = dst[:, h * 32 : (h + 1) * 32, :]
                    i = idx_sb[:, base + (2 * f + h) * IDXC : base + (2 * f + h + 1) * IDXC]
                    outs.append((2 * f + h, d, i))
            return outs

        # queue assignment: q0=g0lo, q1=g0hi, q2=g1lo, q3=g1hi
        @block.gpsimd
        def _(g):
            from concourse import library_config

            g.load_library(library_config.mlp)

            def prep(l):
                for q, d, i in gather_args(l):
                    g.dma_gather(
                        d, src_ap(l), i, HALF, HALF, 128,
                        single_packet=False,
                        prepare_only=True,
                        sem=dsems[q],
                        queue_num=q,
                    ).then_inc(prepsem, 1)

            g.wait_ge(io, 32)  # idx + cnt loaded
            prep(0)
            g.wait_ge(prepsem, 4)
            for q in range(4):
                g.trigger_dma(count=1, queue_num=q)
            for l in range(1, DEV_LAYERS):
                prep(l)  # Q7 descgen overlaps layer l-1 drain/combine/wb
                g.wait_ge(prepsem, 4 * (l + 1))
                g.wait_ge(csem, 2 * l)  # g0/g1 free (combines l-1 done)
                g.wait_ge(wsemA, 16 * l)  # V_l half A written back
                g.wait_ge(wsemB, 16 * l)  # V_l half B written back
                for q in range(4):
                    g.trigger_dma(count=1, queue_num=q)

        mult = mybir.AluOpType.mult
        add = mybir.AluOpType.add

        @block.vector
        def _(v):
            for l in range(DEV_LAYERS):
                op = mult if l % 2 == 0 else add
                # half A: chunks 0..31 (gathers q0 + q2)
                v.wait_ge(dsem0, 16 * (l + 1))
                v.wait_ge(dsem2, 16 * (l + 1))
                if l > 0:
                    v.wait_ge(wsemA, 16 * l)  # wb-A of l-1 done
                v.tensor_tensor(
                    out=comb[:, 0:32, :], in0=g0[:, 0:32, :], in1=g1[:, 0:32, :], op=op
                ).then_inc(csem, 1)
                # half B: chunks 32..63 (gathers q1 + q3)
                v.wait_ge(dsem1, 16 * (l + 1))
                v.wait_ge(dsem3, 16 * (l + 1))
                if l > 0:
                    v.wait_ge(wsemB, 16 * l)  # wb-B of l-1 done
                v.tensor_tensor(
                    out=comb[:, 32:64, :], in0=g0[:, 32:64, :], in1=g1[:, 32:64, :], op=op
                ).then_inc(csem, 1)
            v.wait_ge(psem, 1)
            v.tensor_copy(out=res[:], in_=ps[:]).then_inc(csem, 1)

        @block.sync
        def _(s):
            s.dma_start(idx_sb[:], idxs[:]).then_inc(io, 16)
            s.dma_start(cnt_sb[:], cnt[:]).then_inc(io, 16)
            s.wait_ge(io, 32)
            for l in range(DEV_LAYERS - 1):
                for h, ws in enumerate((wsemA, wsemB)):
                    s.wait_ge(csem, 2 * l + h + 1)
                    s.dma_start(
                        dst_ap(l, h), comb[:, 32 * h : 32 * (h + 1), :]
                    ).then_inc(ws, 16)
            s.wait_ge(csem, 2 * DEV_LAYERS + 1)  # final res copy done
            s.dma_start(out[:], res[:]).then_inc(io, 16)
            s.wait_ge(io, 48)

        @block.tensor
        def _(t):
            t.wait_ge(io, 32)  # cnt loaded
            t.wait_ge(csem, 2 * DEV_LAYERS)  # comb = layer-10 values
            for c in range(CH):
                mm = t.matmul(
                    ps[:],
                    cnt_sb[:, c : c + 1],
                    comb[:, c, :],
                    start=(c == 0),
                    stop=(c == CH - 1),
                )
            mm.then_inc(psem, 1)

    nc.compile()
    return nc


def _get_nc():
    if "nc" not in _CACHE:
        _CACHE["nc"] = _build_nc()
    return _CACHE["nc"]


def _wrap_idx(idx_list):
    return np.tile(idx_list.reshape(-1, 16).T, (8, 1)).astype(np.int16)


def _prep_inputs(weights, neg_weights, children):
    w = np.asarray(weights, np.float32)
    nw = np.asarray(neg_weights, np.float32)
    ch = np.asarray(children, np.int64)

    leaves = np.concatenate([w, nw], axis=1)  # [1024, 8192]

    # write-back permutation: original node j -> row (j%128)*64 + j//128
    def perm(j):
        return (j % 128) * CH + j // 128

    idx_blocks = []
    for l in range(DEV_LAYERS):
        for f in range(2):
            cl = ch[l, :, f]
            if l > 0:
                cl = perm(cl)
            cl = cl.astype(np.int16)
            idx_blocks.append(_wrap_idx(cl[:HALF]))
            idx_blocks.append(_wrap_idx(cl[HALF:]))
    idx_arr = np.ascontiguousarray(np.concatenate(idx_blocks, axis=1))

    count11 = np.bincount(ch[11].ravel(), minlength=WIDTH).astype(np.float32)
    cnt_pc = np.ascontiguousarray(count11.reshape(CH, 128).T)  # [128, 64]

    in_maps = []
    for c in range(N_CORES):
        v0c = np.ascontiguousarray(leaves[c * PB : (c + 1) * PB].T)  # [8192, 128]
        in_maps.append({"v0": v0c, "idxs": idx_arr, "cnt": cnt_pc})
    return in_maps


def run(weights, neg_weights, children, trace=False):
    from concourse.bass_utils import run_bass_kernel_spmd

    nc = _get_nc()
    in_maps = _prep_inputs(weights, neg_weights, children)
    br = run_bass_kernel_spmd(nc, in_maps, list(range(N_CORES)), trace=trace)
    out = np.concatenate([r["out"][0] for r in br.results]).astype(np.float32)
    return out, br


def kernel(weights, neg_weights, children):
    out, _ = run(weights, neg_weights, children)
    return out
